# revision 23
# baseline (speedup 1.0000x reference)
"""Trainium2 Bass kernel for nn_DynamicRoutingRNN.

Strategy (per core, data-parallel over batch 256 -> 8 x 32):
  The scan  y_{t+1} = L*y_t + 0.2*relu(y_t@W^T + d_t) + n_t   (L=0.8)
  is rewritten in "pre-activation" space:  pre_{t+1} = L*pre_t + s_t@W^T + c_t,
  with s_t = 0.2*relu(pre_t) + n_t and c_t = d_{t+1} - L*d_t (rank<=8).
  Chunk-locally rescaled state Q_j = pre_{t0+j}/L^j turns the leak into a pure
  accumulation that lives entirely in PSUM:
      Q_{j+1} = Q_j + 0.25*S'_j@W^T + c'_j ,  S'_j = relu(Q_j) + 4*n_j/L^{j+1}
  The state is kept TRANSPOSED (n on partitions), so each step is just:
      evac: S' = relu(psum) + noise'   (DVE fused op / ACT relu)
      6 small accumulating matmuls with constant stationary weights.
  y_seq is reconstructed afterwards (chunk-batched) as a pure cumsum of S'
  (tensor_tensor_scan), un-transposed via PE, and z = sigmoid(y.w_out+b) via a
  fused multiply+accumulate.  Noise is pre-scaled/transposed via gpsimd casts +
  DMA-transpose through a DRAM bounce, overlapped with the scan.
"""
import os
import numpy as np
import ml_dtypes

import concourse.bass as bass
import concourse.mybir as mybir
import concourse.tile as tile
from concourse import bacc, bass_utils
from concourse.masks import make_identity

FP32 = mybir.dt.float32
BF16 = mybir.dt.bfloat16
AL = mybir.AluOpType
AF = mybir.ActivationFunctionType

N = 200
NI = 7
B = 32           # per-core batch
NCORES = 8
L = 0.8
ALPHA = 0.2
NOISE_SCALE = 0.15 * float(np.sqrt(np.float32(2.0 * 0.2)))
TC = 128         # scan chunk length
K1 = 128         # contraction rows n in [0,128)
K2 = 72          # contraction rows n in [128,200)


def _chunks(T):
    out = []
    t0 = 0
    while t0 < T:
        out.append((t0, min(TC, T - t0)))
        t0 += TC
    return out


def build_program(T):
    chunks = _chunks(T)
    nc = bacc.Bacc("TRN2", target_bir_lowering=False, debug=False,
                   num_devices=NCORES)

    # ---------------- DRAM I/O ----------------
    y0_d = nc.dram_tensor("y0", (B, N), FP32, kind="ExternalInput")
    u_d = nc.dram_tensor("u", (B, T, NI), FP32, kind="ExternalInput")
    noise_d = nc.dram_tensor("noise", (B, T, N), FP32, kind="ExternalInput")
    cb1a_d = nc.dram_tensor("cb1a", (K1, 128), BF16, kind="ExternalInput")
    cb1b_d = nc.dram_tensor("cb1b", (K1, K2), BF16, kind="ExternalInput")
    cb2a_d = nc.dram_tensor("cb2a", (128, 128), BF16, kind="ExternalInput")
    cb2b_d = nc.dram_tensor("cb2b", (128, K2), BF16, kind="ExternalInput")
    nstab_d = nc.dram_tensor("nstab", (128, TC * B), BF16, kind="ExternalInput")
    ktab_d = nc.dram_tensor("ktab", (B, TC * 8), FP32, kind="ExternalInput")
    scol_d = nc.dram_tensor("scol", (128, 1), FP32, kind="ExternalInput")
    wrep_d = nc.dram_tensor("wrep", (128, N), FP32, kind="ExternalInput")

    ys_d = nc.dram_tensor("y_seq", (B, T, N), FP32, kind="ExternalOutput")
    zs_d = nc.dram_tensor("z_seq", (B, T, 1), FP32, kind="ExternalOutput")
    yf_d = nc.dram_tensor("y_final", (B, N), FP32, kind="ExternalOutput")

    # 256 cols: n in [0,200) + zero pad — keeps both DMA-transposes 128-wide
    # and makes the second transpose deposit zeros at partitions 72:128.
    # cols: 0:200 noise(bf16), 224:232 drive stream u', rest zeros.  The
    # 128-col DMA-transpose of cols 128:256 then yields the chunk-2 matmul rhs
    # directly: rows 0:72 noise, 96:104 drive, rest zeros.
    nbt_d = nc.dram_tensor("nbt", (T * B, 256), BF16, kind="Internal")

    # ---------------- persistent SBUF ----------------
    us = nc.alloc_sbuf_tensor("us", [B, T * NI], FP32).ap()
    n1b = [nc.alloc_sbuf_tensor(f"n1b{i}", [128, TC * B], BF16).ap() for i in range(2)]
    n2b = [nc.alloc_sbuf_tensor(f"n2b{i}", [128, TC * B], BF16).ap() for i in range(2)]
    s1b = [nc.alloc_sbuf_tensor(f"s1b{i}", [128, TC * B], BF16).ap() for i in range(2)]
    s2b = [nc.alloc_sbuf_tensor(f"s2b{i}", [128, TC * B], BF16).ap() for i in range(2)]
    yt1 = nc.alloc_sbuf_tensor("yt1", [128, TC * B], FP32).ap()
    yt2 = nc.alloc_sbuf_tensor("yt2", [128, TC * B], FP32).ap()
    nstab = nc.alloc_sbuf_tensor("nstab_s", [128, TC * B], BF16).ap()
    ktab = nc.alloc_sbuf_tensor("ktab_s", [B, TC * 8], FP32).ap()
    scol = nc.alloc_sbuf_tensor("scol_s", [128, 1], FP32).ap()
    wrep = nc.alloc_sbuf_tensor("wrep_s", [128, N], FP32).ap()
    ones = nc.alloc_sbuf_tensor("ones_s", [128, TC], FP32).ap()
    id128 = nc.alloc_sbuf_tensor("id128", [128, 128], FP32).ap()
    cb1a = nc.alloc_sbuf_tensor("cb1a_s", [K1, 128], BF16).ap()
    cb1b = nc.alloc_sbuf_tensor("cb1b_s", [K1, K2], BF16).ap()
    cb2a = nc.alloc_sbuf_tensor("cb2a_s", [128, 128], BF16).ap()
    cb2b = nc.alloc_sbuf_tensor("cb2b_s", [128, K2], BF16).ap()
    carryA = nc.alloc_sbuf_tensor("carryA", [128, B], FP32).ap()
    carryB = nc.alloc_sbuf_tensor("carryB", [128, B], FP32).ap()
    utile = [nc.alloc_sbuf_tensor(f"utile{i}", [B, TC * 8], FP32).ap() for i in range(2)]
    ubf = [nc.alloc_sbuf_tensor(f"ubf{i}", [B, TC * 8], BF16).ap() for i in range(2)]
    zblk = [nc.alloc_sbuf_tensor(f"zblk{i}", [128, B], FP32).ap() for i in range(2)]
    co = [nc.alloc_sbuf_tensor(f"co{i}", [128, 4096], BF16).ap() for i in range(2)]
    y0s = nc.alloc_sbuf_tensor("y0s", [B, N], FP32).ap()
    u0aug = nc.alloc_sbuf_tensor("u0aug", [B, 8], FP32).ap()
    init2 = nc.alloc_sbuf_tensor("init2", [128, B], BF16).ap()
    y0t4a = nc.alloc_sbuf_tensor("y0t4a", [128, B], BF16).ap()
    junk = nc.alloc_sbuf_tensor("junk", [128, N], FP32).ap()

    with tile.TileContext(nc) as tc:
        with (
            tc.tile_pool(name="pA", bufs=1, space=bass.MemorySpace.PSUM) as pA,
            tc.tile_pool(name="pB", bufs=1, space=bass.MemorySpace.PSUM) as pB,
            tc.tile_pool(name="pst", bufs=2, space=bass.MemorySpace.PSUM) as pstp,
            tc.tile_pool(name="psi", bufs=1, space=bass.MemorySpace.PSUM) as psip,
            tc.tile_pool(name="psz", bufs=2, space=bass.MemorySpace.PSUM) as pszp,
            tc.tile_pool(name="cast", bufs=2) as castp,
            tc.tile_pool(name="ynat", bufs=4) as ynatp,
            tc.tile_pool(name="zt", bufs=2) as ztp,
        ):
            psA = pA.tile([128, B], FP32)
            psB = pB.tile([128, B], FP32)

            # ======== init / constants ========
            make_identity(nc, id128)
            nc.gpsimd.memset(ones, 1.0)
            for i in range(2):
                nc.gpsimd.memset(s2b[i][64:128, :], 0.0)
                nc.gpsimd.memset(co[i], 0.0)
            nc.sync.dma_start(us, u_d.ap().rearrange("b t i -> b (t i)"))
            nc.sync.dma_start(y0s, y0_d.ap())
            nc.sync.dma_start(nstab, nstab_d.ap())
            nc.sync.dma_start(ktab, ktab_d.ap())
            nc.sync.dma_start(scol, scol_d.ap())
            nc.sync.dma_start(wrep, wrep_d.ap())
            nc.sync.dma_start(cb1a, cb1a_d.ap())
            nc.sync.dma_start(cb1b, cb1b_d.ap())
            nc.sync.dma_start(cb2a, cb2a_d.ap())
            nc.sync.dma_start(cb2b, cb2b_d.ap())

            # one-time zero fill of nbt pad cols (200:256) for all rows:
            # supplies the zero rows 80:128 of the chunk-2 rhs (and the unused
            # drive slot of the final step).  Uses the zeroed co[0] as source.
            r0 = 0
            while r0 < T * B:
                rr = min(4096, T * B - r0)
                assert rr * 56 % 128 == 0
                nc.sync.dma_start(nbt_d.ap()[r0:r0 + rr, 200:256],
                                  co[0][:, 0:rr * 56 // 128])
                r0 += rr

            # u0aug = [u[:,0,:], 1.0]; feed -u0aug via sign flip later
            nc.vector.tensor_copy(u0aug[:, 0:NI], us[:, 0:NI])
            nc.vector.memset(u0aug[:, NI:8], 1.0)
            nc.vector.tensor_scalar_mul(u0aug, u0aug, -1.0)

            # transposes: psi = [y0T_a (128,32) | y0T_b (72,32) | u0T (8,32)]
            psi = psip.tile([128, 96], FP32)
            nc.tensor.transpose(psi[:, 0:32], y0s[:, 0:K1], id128[0:B, 0:B])
            nc.tensor.transpose(psi[0:K2, 32:64], y0s[:, K1:N], id128[0:B, 0:B])
            nc.tensor.transpose(psi[0:8, 64:96], u0aug, id128[0:B, 0:B])
            nc.vector.tensor_scalar_mul(y0t4a, psi[:, 0:32], 4.0)
            nc.vector.memset(init2, 0.0)
            nc.vector.tensor_scalar_mul(init2[0:K2, :], psi[0:K2, 32:64], 4.0)
            nc.vector.tensor_copy(init2[96:104, :], psi[0:8, 64:96])
            nc.vector.tensor_scalar_mul(carryA, psi[:, 0:32], 4.0)
            nc.vector.tensor_scalar_mul(carryB[0:K2, :], psi[0:K2, 32:64], 4.0)

            # init matmuls: psum <- pre_0^T (Q_0)
            nc.tensor.matmul(psA[:], cb1a, y0t4a, start=True, stop=False)
            nc.tensor.matmul(psA[:], cb2a, init2, start=False, stop=False)
            nc.tensor.matmul(psB[0:K2, :], cb1b, y0t4a, start=True, stop=False)
            nc.tensor.matmul(psB[0:K2, :], cb2b, init2, start=False, stop=False)

            # ======== per-chunk emission helpers ========
            def pre_pass_items(ci):
                """Emit-thunks preparing chunk ci's noise/drive streams."""
                t0, tc_ = chunks[ci]
                cur = ci % 2
                items = []
                # halves for pipelining the cast
                hsizes = []
                h0 = min(64, tc_)
                hsizes.append(h0)
                if tc_ > h0:
                    hsizes.append(tc_ - h0)
                hoff = 0
                for hi, hs in enumerate(hsizes):
                    th0 = t0 + hoff
                    fh = hs * 200 // 4  # free size of cast-in tile
                    col0 = hoff * B

                    def do_cast(th0=th0, hs=hs, fh=fh, hi=hi):
                        ci_t = castp.tile([128, 3200], FP32, tag="castin")
                        co_t = co[hi]
                        src = noise_d.ap()[:, th0:th0 + hs, :].rearrange(
                            "b (q t) n -> b q t n", q=4)
                        nc.sync.dma_start(ci_t[:, 0:fh], src)
                        nc.gpsimd.tensor_copy(co_t[:, 0:fh], ci_t[:, 0:fh])
                        dst = nbt_d.ap()[th0 * B:(th0 + hs) * B, 0:200].rearrange(
                            "(q t b) n -> b q t n", q=4, b=B)
                        nc.sync.dma_start(dst, co_t[:, 0:fh])

                    def do_tr(th0=th0, hs=hs, col0=col0, cur=cur):
                        rows = nbt_d.ap()[th0 * B:(th0 + hs) * B, :]
                        nc.sync.dma_start_transpose(
                            n1b[cur][:, col0:col0 + hs * B], rows[:, 0:K1])
                        nc.sync.dma_start_transpose(
                            n2b[cur][:, col0:col0 + hs * B], rows[:, K1:256])

                    def do_scale(col0=col0, hs=hs, cur=cur):
                        cs = slice(col0, col0 + hs * B)
                        nc.gpsimd.tensor_tensor(
                            n1b[cur][:, cs], n1b[cur][:, cs], nstab[:, cs], AL.mult)
                        nc.gpsimd.tensor_tensor(
                            n2b[cur][0:K2, cs], n2b[cur][0:K2, cs],
                            nstab[0:K2, cs], AL.mult)

                    items += [do_cast, do_tr, do_scale]
                    hoff += hs


                # drive stream (u-combos) for this chunk -> nbt cols 200:208
                tcu = tc_ if t0 + tc_ < T else tc_ - 1  # steps with a next-u
                if tcu > 0:
                    def do_u(ci=ci, t0=t0, tcu=tcu, cur=cur):
                        ut, ub = utile[cur], ubf[cur]
                        # utile[:, j*8+i] (i<7) = 0.8*u[t0+j] - u[t0+j+1]
                        o_ap = ut[:, 0:tcu * 8].rearrange("b (j i) -> b j i", i=8)[:, :, 0:7]
                        i0 = us[:, t0 * 7:(t0 + tcu) * 7].rearrange("b (j i) -> b j i", i=7)
                        i1 = us[:, (t0 + 1) * 7:(t0 + tcu + 1) * 7].rearrange("b (j i) -> b j i", i=7)
                        nc.vector.scalar_tensor_tensor(o_ap, i0, L, i1, AL.mult, AL.subtract)
                        nc.gpsimd.memset(
                            ut[:, 0:tcu * 8].rearrange("b (j i) -> b j i", i=8)[:, :, 7:8],
                            -ALPHA)
                        nc.gpsimd.tensor_tensor(ub[:, 0:tcu * 8], ut[:, 0:tcu * 8],
                                                ktab[:, 0:tcu * 8], AL.mult)
                        dst = nbt_d.ap()[t0 * B:(t0 + tcu) * B, 224:232].rearrange(
                            "(j b) i -> b j i", b=B)
                        nc.sync.dma_start(dst, ub[:, 0:tcu * 8])
                    items.insert(0, do_u)
                return items

            def post_pass_items(ci):
                """Emit-thunks for reconstruction of chunk ci (after its scan)."""
                t0, tc_ = chunks[ci]
                cur = ci % 2
                last = ci == len(chunks) - 1
                items = []

                def do_s2tot(cur=cur, tc_=tc_):
                    cs = slice(0, tc_ * B)
                    nc.gpsimd.tensor_tensor(s2b[cur][0:K2, cs], s2b[cur][0:K2, cs],
                                            n2b[cur][0:K2, cs], AL.add)
                items.append(do_s2tot)

                # cumsum: scan over j per b; output laid b-major (col b*TC+j)
                s1v = s1b[cur][:, 0:tc_ * B].rearrange("p (j b) -> p b j", b=B)
                s2v = s2b[cur][0:K2, 0:tc_ * B].rearrange("p (j b) -> p b j", b=B)
                for b in range(B):
                    def do_scan(b=b, tc_=tc_):
                        nc.vector.tensor_tensor_scan(
                            yt1[:, b * TC:b * TC + tc_], ones[:, 0:tc_],
                            s1v[:, b, :], carryA[:, b:b + 1], AL.mult, AL.add)
                        nc.vector.tensor_tensor_scan(
                            yt2[0:K2, b * TC:b * TC + tc_], ones[0:K2, 0:tc_],
                            s2v[:, b, :], carryB[0:K2, b:b + 1], AL.mult, AL.add)
                    items.append(do_scan)

                def do_carry(tc_=tc_):
                    f = float(L ** tc_)
                    v1 = yt1[:, 0:B * TC].rearrange("p (b j) -> p j b", j=TC)
                    v2 = yt2[0:K2, 0:B * TC].rearrange("p (b j) -> p j b", j=TC)
                    nc.gpsimd.tensor_scalar_mul(carryA, v1[:, tc_ - 1, :], f)
                    nc.gpsimd.tensor_scalar_mul(carryB[0:K2, :], v2[:, tc_ - 1, :], f)
                items.append(do_carry)

                # un-transpose per batch row: (n, j) -> (j, n), scale, z-dot
                for b in range(B):
                    def do_block(b=b, t0=t0, tc_=tc_, ci=ci):
                        cs = slice(b * TC, b * TC + tc_)
                        pst = pstp.tile([128, N], FP32, tag="pst")
                        nc.tensor.matmul(pst[0:tc_, 0:K1], yt1[:, cs], id128,
                                         is_transpose=True, start=True,
                                         stop=False)
                        nc.tensor.matmul(pst[0:tc_, K1:N], yt2[0:K2, cs],
                                         id128[0:K2, 0:K2], is_transpose=True,
                                         start=False, stop=True)
                        yn = ynatp.tile([128, N], FP32, tag="ynat")
                        nc.vector.tensor_scalar(yn[0:tc_, :], pst[0:tc_, :],
                                                scol[0:tc_, :], None, AL.mult)
                        nc.vector.scalar_tensor_tensor(
                            junk[0:tc_, :], yn[0:tc_, :], 1.0, wrep[0:tc_, :],
                            AL.mult, AL.mult,
                            accum_out=zblk[ci % 2][0:tc_, b:b + 1])
                        nc.sync.dma_start(ys_d.ap()[b, t0:t0 + tc_, :], yn[0:tc_, :])
                    items.append(do_block)

                def do_z(ci=ci, t0=t0, tc_=tc_):
                    psz = pszp.tile([B, 128], FP32, tag="psz")
                    nc.tensor.matmul(psz[:, 0:tc_], zblk[ci % 2][0:tc_, :],
                                     id128[0:tc_, 0:tc_], is_transpose=True)
                    zt = ztp.tile([B, TC], FP32, tag="zt")
                    nc.scalar.activation(zt[:, 0:tc_], psz[:, 0:tc_],
                                         AF.Sigmoid, bias=_BOUT[0])
                    nc.sync.dma_start(
                        zs_d.ap()[:, t0:t0 + tc_, :].rearrange("b t o -> b (t o)"),
                        zt[:, 0:tc_])
                items.append(do_z)

                if last:
                    def do_yf():
                        nc.sync.dma_start(yf_d.ap(), ys_d.ap()[:, T - 1, :])
                    items.append(do_yf)
                return items

            # first transpose of each untranspose-pair opens the psum group
            # patched below via explicit matmul call; see do_block (uses
            # transpose for first, matmul(is_transpose) for second).

            # ======== main emission loop ========
            pending = pre_pass_items(0)
            for it in pending:
                it()

            for ci, (t0, tc_) in enumerate(chunks):
                cur = ci % 2
                items = []
                if ci > 0:
                    items += post_pass_items(ci - 1)
                if ci + 1 < len(chunks):
                    items += pre_pass_items(ci + 1)

                if ci > 0:
                    ptc = chunks[ci - 1][1]
                    f = float(L ** ptc)
                    nc.vector.tensor_scalar_mul(psA[:], psA[:], f)
                    nc.vector.tensor_scalar_mul(psB[0:K2, :], psB[0:K2, :], f)

                ndone = 0
                for j in range(tc_):
                    t = t0 + j
                    js = slice(j * B, (j + 1) * B)
                    nc.vector.scalar_tensor_tensor(
                        s1b[cur][:, js], psA[:], 0.0, n1b[cur][:, js],
                        AL.max, AL.add)
                    nc.scalar.activation(s2b[cur][0:K2, js], psB[0:K2, :], AF.Relu)
                    if t < T - 1:
                        stop = t == T - 2
                        nc.tensor.matmul(psA[:], cb1a, s1b[cur][:, js],
                                         start=False, stop=False)
                        nc.tensor.matmul(psA[:], cb2a, s2b[cur][:, js],
                                         start=False, stop=False)
                        nc.tensor.matmul(psA[:], cb2a, n2b[cur][:, js],
                                         start=False, stop=stop)
                        nc.tensor.matmul(psB[0:K2, :], cb1b, s1b[cur][:, js],
                                         start=False, stop=False)
                        nc.tensor.matmul(psB[0:K2, :], cb2b, s2b[cur][:, js],
                                         start=False, stop=False)
                        nc.tensor.matmul(psB[0:K2, :], cb2b, n2b[cur][:, js],
                                         start=False, stop=stop)
                    # interleave deferred work
                    want = (j + 1) * len(items) // tc_
                    while ndone < want:
                        items[ndone]()
                        ndone += 1
                while ndone < len(items):
                    items[ndone]()
                    ndone += 1

            # final chunk's post pass
            for it in post_pass_items(len(chunks) - 1):
                it()

    nc.compile()
    return nc


_BOUT = [0.0]  # patched by kernel() before build (bias baked into program)


def _host_constants(W_in_raw, W_rec, b_rec, w_out):
    bf = ml_dtypes.bfloat16
    W_in = np.abs(W_in_raw.astype(np.float32))
    Wa = np.concatenate([W_in, b_rec.astype(np.float32)[:, None]], axis=1)  # (N,8)
    C = (0.25 * W_rec.astype(np.float32).T)  # [k, n]
    cb1a = C[0:K1, 0:K1].astype(bf)
    cb1b = C[0:K1, K1:N].astype(bf)
    cb2a = np.zeros((128, 128), bf)
    cb2b = np.zeros((128, K2), bf)
    cb2a[0:K2, :] = C[K1:N, 0:K1].astype(bf)
    cb2b[0:K2, :] = C[K1:N, K1:N].astype(bf)
    cb2a[96:104, :] = (-Wa.T[:, 0:K1]).astype(bf)
    cb2b[96:104, :] = (-Wa.T[:, K1:N]).astype(bf)

    j = np.arange(TC, dtype=np.float64)
    nst = (4.0 * NOISE_SCALE * L ** (-(j + 1.0))).astype(np.float32)
    nstab = np.broadcast_to(np.repeat(nst, B)[None, :], (128, TC * B)).astype(bf)
    ktab = np.broadcast_to(np.repeat((L ** (-(j + 1.0))).astype(np.float32), 8)[None, :],
                           (B, TC * 8)).astype(np.float32).copy()
    scolm = (0.25 * L ** (np.arange(128) + 1.0)).astype(np.float32)[:, None]
    wrep = np.broadcast_to(w_out.astype(np.float32)[0][None, :], (128, N)).copy()
    return dict(cb1a=np.ascontiguousarray(cb1a), cb1b=np.ascontiguousarray(cb1b),
                cb2a=cb2a, cb2b=cb2b, nstab=np.ascontiguousarray(nstab),
                ktab=ktab, scol=np.ascontiguousarray(scolm), wrep=wrep)


_prog_cache = {}


def _get_program(T, bout):
    key = (T, float(bout))
    if key not in _prog_cache:
        _BOUT[0] = float(bout)
        _prog_cache[key] = build_program(T)
    return _prog_cache[key]


def kernel(y0, u_sequence, noise, W_in_raw, W_rec, b_rec, w_out, b_out,
           run_kwargs=None):
    T = u_sequence.shape[1]
    Bfull = y0.shape[0]
    assert Bfull == B * NCORES
    nc = _get_program(T, float(np.asarray(b_out).reshape(-1)[0]))
    consts = _host_constants(W_in_raw, W_rec, b_rec, w_out)

    in_maps = []
    for c in range(NCORES):
        sl = slice(c * B, (c + 1) * B)
        m = dict(consts)
        m["y0"] = np.ascontiguousarray(y0[sl].astype(np.float32))
        m["u"] = np.ascontiguousarray(u_sequence[sl].astype(np.float32))
        m["noise"] = np.ascontiguousarray(noise[sl].astype(np.float32))
        in_maps.append(m)

    res = bass_utils.run_bass_kernel_spmd(
        nc, in_maps, core_ids=list(range(NCORES)), **(run_kwargs or {}))

    y_seq = np.concatenate([res.results[c]["y_seq"] for c in range(NCORES)], axis=0)
    z_seq = np.concatenate([res.results[c]["z_seq"] for c in range(NCORES)], axis=0)
    y_fin = np.concatenate([res.results[c]["y_final"] for c in range(NCORES)], axis=0)
    kernel.last_result = res
    return y_seq, z_seq, y_fin


# revision 30
# speedup vs baseline: 1.4332x; 1.4332x over previous
"""Trainium2 Bass kernel for nn_DynamicRoutingRNN.

Strategy (per core, data-parallel over batch 256 -> 8 x 32):
  The scan  y_{t+1} = L*y_t + 0.2*relu(y_t@W^T + d_t) + n_t   (L=0.8)
  is rewritten in "pre-activation" space:  pre_{t+1} = L*pre_t + s_t@W^T + c_t,
  with s_t = 0.2*relu(pre_t) + n_t and c_t = d_{t+1} - L*d_t (rank<=8).
  Chunk-locally rescaled state Q_j = pre_{t0+j}/L^j turns the leak into a pure
  accumulation that lives entirely in PSUM:
      Q_{j+1} = Q_j + 0.25*S'_j@W^T + c'_j ,  S'_j = relu(Q_j) + 4*n_j/L^{j+1}
  The state is kept TRANSPOSED (n on partitions), so each step is just:
      evac: S' = relu(psum) + noise'   (DVE fused op / ACT relu)
      6 small accumulating matmuls with constant stationary weights.
  y_seq is reconstructed afterwards (chunk-batched) as a pure cumsum of S'
  (tensor_tensor_scan), un-transposed via PE, and z = sigmoid(y.w_out+b) via a
  fused multiply+accumulate.  Noise is pre-scaled/transposed via gpsimd casts +
  DMA-transpose through a DRAM bounce, overlapped with the scan.
"""
import os
import numpy as np
import ml_dtypes

import concourse.bass as bass
import concourse.mybir as mybir
import concourse.tile as tile
from concourse import bacc, bass_utils
from concourse.masks import make_identity

FP32 = mybir.dt.float32
BF16 = mybir.dt.bfloat16
AL = mybir.AluOpType
AF = mybir.ActivationFunctionType

N = 200
NI = 7
B = 32           # per-core batch
NCORES = 8
L = 0.8
ALPHA = 0.2
NOISE_SCALE = 0.15 * float(np.sqrt(np.float32(2.0 * 0.2)))
TC = 128         # scan chunk length
K1 = 128         # contraction rows n in [0,128)
K2 = 72          # contraction rows n in [128,200)


def _chunks(T):
    out = []
    t0 = 0
    while t0 < T:
        out.append((t0, min(TC, T - t0)))
        t0 += TC
    return out


def build_program(T):
    chunks = _chunks(T)
    nc = bacc.Bacc("TRN2", target_bir_lowering=False, debug=False,
                   num_devices=NCORES)

    # ---------------- DRAM I/O ----------------
    y0_d = nc.dram_tensor("y0", (B, N), FP32, kind="ExternalInput")
    u_d = nc.dram_tensor("u", (B, T, NI), FP32, kind="ExternalInput")
    noise_d = nc.dram_tensor("noise", (B, T, N), FP32, kind="ExternalInput")
    cb1a_d = nc.dram_tensor("cb1a", (K1, 128), BF16, kind="ExternalInput")
    cb1b_d = nc.dram_tensor("cb1b", (K1, K2), BF16, kind="ExternalInput")
    cb2a_d = nc.dram_tensor("cb2a", (128, 128), BF16, kind="ExternalInput")
    cb2b_d = nc.dram_tensor("cb2b", (128, K2), BF16, kind="ExternalInput")
    nstab_d = nc.dram_tensor("nstab", (128, TC * B), BF16, kind="ExternalInput")
    ktab_d = nc.dram_tensor("ktab", (B, TC * 8), FP32, kind="ExternalInput")
    scol_d = nc.dram_tensor("scol", (128, 1), FP32, kind="ExternalInput")
    wrep_d = nc.dram_tensor("wrep", (128, N), FP32, kind="ExternalInput")
    lt_d = nc.dram_tensor("lt", (128, 128), BF16, kind="ExternalInput")

    ys_d = nc.dram_tensor("y_seq", (B, T, N), FP32, kind="ExternalOutput")
    zs_d = nc.dram_tensor("z_seq", (B, T, 1), FP32, kind="ExternalOutput")
    yf_d = nc.dram_tensor("y_final", (B, N), FP32, kind="ExternalOutput")

    # 256 cols: n in [0,200) + zero pad — keeps both DMA-transposes 128-wide
    # and makes the second transpose deposit zeros at partitions 72:128.
    # cols: 0:200 noise(bf16), 224:232 drive stream u', rest zeros.  The
    # 128-col DMA-transpose of cols 128:256 then yields the chunk-2 matmul rhs
    # directly: rows 0:72 noise, 96:104 drive, rest zeros.
    nbt_d = nc.dram_tensor("nbt", (T * B, 256), BF16, kind="Internal")

    # ---------------- persistent SBUF ----------------
    us = nc.alloc_sbuf_tensor("us", [B, T * NI], FP32).ap()
    n1b = [nc.alloc_sbuf_tensor(f"n1b{i}", [128, TC * B], BF16).ap() for i in range(2)]
    n2b = [nc.alloc_sbuf_tensor(f"n2b{i}", [128, TC * B], BF16).ap() for i in range(2)]
    s1b = [nc.alloc_sbuf_tensor(f"s1b{i}", [128, TC * B], BF16).ap() for i in range(2)]
    s2b = [nc.alloc_sbuf_tensor(f"s2b{i}", [128, TC * B], BF16).ap() for i in range(2)]
    nstab = nc.alloc_sbuf_tensor("nstab_s", [128, TC * B], BF16).ap()
    ktab = nc.alloc_sbuf_tensor("ktab_s", [B, TC * 8], FP32).ap()
    scol = nc.alloc_sbuf_tensor("scol_s", [128, 1], FP32).ap()
    wrep = nc.alloc_sbuf_tensor("wrep_s", [128, N], FP32).ap()
    id128 = nc.alloc_sbuf_tensor("id128", [128, 128], FP32).ap()
    id128b = nc.alloc_sbuf_tensor("id128b", [128, 128], BF16).ap()
    cb1a = nc.alloc_sbuf_tensor("cb1a_s", [K1, 128], BF16).ap()
    cb1b = nc.alloc_sbuf_tensor("cb1b_s", [K1, K2], BF16).ap()
    cb2a = nc.alloc_sbuf_tensor("cb2a_s", [128, 128], BF16).ap()
    cb2b = nc.alloc_sbuf_tensor("cb2b_s", [128, K2], BF16).ap()
    lt = nc.alloc_sbuf_tensor("lt_s", [128, 128], BF16).ap()
    c4 = nc.alloc_sbuf_tensor("c4_s", [1, 128], BF16).ap()
    ycar = nc.alloc_sbuf_tensor("ycar", [1, B * N], BF16).ap()
    utile = [nc.alloc_sbuf_tensor(f"utile{i}", [B, TC * 8], FP32).ap() for i in range(2)]
    ubf = [nc.alloc_sbuf_tensor(f"ubf{i}", [B, TC * 8], BF16).ap() for i in range(2)]
    zblk = [nc.alloc_sbuf_tensor(f"zblk{i}", [128, B], FP32).ap() for i in range(2)]
    co = [nc.alloc_sbuf_tensor(f"co{i}", [128, 4096], BF16).ap() for i in range(2)]
    y0s = nc.alloc_sbuf_tensor("y0s", [B, N], FP32).ap()
    u0aug = nc.alloc_sbuf_tensor("u0aug", [B, 8], FP32).ap()
    init2 = nc.alloc_sbuf_tensor("init2", [128, B], BF16).ap()
    y0t4a = nc.alloc_sbuf_tensor("y0t4a", [128, B], BF16).ap()
    junk = nc.alloc_sbuf_tensor("junk", [128, N], FP32).ap()

    with tile.TileContext(nc) as tc:
        with (
            tc.tile_pool(name="pA", bufs=1, space=bass.MemorySpace.PSUM) as pA,
            tc.tile_pool(name="pB", bufs=1, space=bass.MemorySpace.PSUM) as pB,
            tc.tile_pool(name="pst", bufs=2, space=bass.MemorySpace.PSUM) as pstp,
            tc.tile_pool(name="p2", bufs=2, space=bass.MemorySpace.PSUM) as p2p,
            tc.tile_pool(name="pstb", bufs=2, space=bass.MemorySpace.PSUM) as pstbp,
            tc.tile_pool(name="ynat", bufs=4) as ynatp,
            tc.tile_pool(name="snat", bufs=3) as snatp,
            tc.tile_pool(name="zt", bufs=2) as ztp,
        ):
            psA = pA.tile([128, B], FP32)
            psB = pB.tile([128, B], FP32)

            # ======== init / constants ========
            make_identity(nc, id128)
            nc.gpsimd.tensor_copy(id128b, id128)
            nc.gpsimd.memset(c4, 4.0)
            for i in range(2):
                nc.gpsimd.memset(s2b[i][64:128, :], 0.0)
                nc.gpsimd.memset(co[i], 0.0)
            nc.sync.dma_start(us, u_d.ap().rearrange("b t i -> b (t i)"))
            nc.sync.dma_start(y0s, y0_d.ap())
            nc.sync.dma_start(nstab, nstab_d.ap())
            nc.sync.dma_start(ktab, ktab_d.ap())
            nc.sync.dma_start(scol, scol_d.ap())
            nc.sync.dma_start(wrep, wrep_d.ap())
            nc.sync.dma_start(cb1a, cb1a_d.ap())
            nc.sync.dma_start(cb1b, cb1b_d.ap())
            nc.sync.dma_start(cb2a, cb2a_d.ap())
            nc.sync.dma_start(cb2b, cb2b_d.ap())
            nc.sync.dma_start(lt, lt_d.ap())
            nc.gpsimd.dma_start(ycar, y0_d.ap())  # casting DMA f32->bf16

            # one-time zero fill of nbt pad cols (200:256) for all rows:
            # supplies the zero rows 80:128 of the chunk-2 rhs (and the unused
            # drive slot of the final step).  Uses the zeroed co[0] as source.
            r0 = 0
            while r0 < T * B:
                rr = min(4096, T * B - r0)
                assert rr * 56 % 128 == 0
                nc.sync.dma_start(nbt_d.ap()[r0:r0 + rr, 200:256],
                                  co[0][:, 0:rr * 56 // 128])
                r0 += rr

            # u0aug = [u[:,0,:], 1.0]; feed -u0aug via sign flip later
            nc.vector.tensor_copy(u0aug[:, 0:NI], us[:, 0:NI])
            nc.vector.memset(u0aug[:, NI:8], 1.0)
            nc.vector.tensor_scalar_mul(u0aug, u0aug, -1.0)

            # transposes: psi = [y0T_a (128,32) | y0T_b (72,32) | u0T (8,32)]
            psi_t = pstp.tile([128, N], FP32, tag="pst", name="psi")
            psi = psi_t[:, 0:96]
            nc.tensor.transpose(psi[:, 0:32], y0s[:, 0:K1], id128[0:B, 0:B])
            nc.tensor.transpose(psi[0:K2, 32:64], y0s[:, K1:N], id128[0:B, 0:B])
            nc.tensor.transpose(psi[0:8, 64:96], u0aug, id128[0:B, 0:B])
            nc.vector.tensor_scalar_mul(y0t4a, psi[:, 0:32], 4.0)
            nc.vector.memset(init2, 0.0)
            nc.vector.tensor_scalar_mul(init2[0:K2, :], psi[0:K2, 32:64], 4.0)
            nc.vector.tensor_copy(init2[96:104, :], psi[0:8, 64:96])

            # init matmuls: psum <- pre_0^T (Q_0)
            nc.tensor.matmul(psA[:], cb1a, y0t4a, start=True, stop=False)
            nc.tensor.matmul(psA[:], cb2a, init2, start=False, stop=False)
            nc.tensor.matmul(psB[0:K2, :], cb1b, y0t4a, start=True, stop=False)
            nc.tensor.matmul(psB[0:K2, :], cb2b, init2, start=False, stop=False)

            # ======== per-chunk emission helpers ========
            def pre_pass_items(ci):
                """Emit-thunks preparing chunk ci's noise/drive streams."""
                t0, tc_ = chunks[ci]
                cur = ci % 2
                items = []
                # halves for pipelining the cast
                hsizes = []
                h0 = min(64, tc_)
                hsizes.append(h0)
                if tc_ > h0:
                    hsizes.append(tc_ - h0)
                hoff = 0
                for hi, hs in enumerate(hsizes):
                    th0 = t0 + hoff
                    fh = hs * 200 // 4  # free size of cast-in tile
                    col0 = hoff * B

                    def do_cast(th0=th0, hs=hs):
                        dst = nbt_d.ap()[th0 * B:(th0 + hs) * B, 0:200].rearrange(
                            "(j b) n -> b j n", b=B)
                        nc.gpsimd.dma_start(dst, noise_d.ap()[:, th0:th0 + hs, :])

                    def do_tr(th0=th0, hs=hs, col0=col0, cur=cur):
                        rows = nbt_d.ap()[th0 * B:(th0 + hs) * B, :]
                        nc.sync.dma_start_transpose(
                            n1b[cur][:, col0:col0 + hs * B], rows[:, 0:K1])
                        nc.sync.dma_start_transpose(
                            n2b[cur][:, col0:col0 + hs * B], rows[:, K1:256])

                    def do_scale(col0=col0, hs=hs, cur=cur):
                        cs = slice(col0, col0 + hs * B)
                        nc.gpsimd.tensor_tensor(
                            n1b[cur][:, cs], n1b[cur][:, cs], nstab[:, cs], AL.mult)
                        nc.gpsimd.tensor_tensor(
                            n2b[cur][0:K2, cs], n2b[cur][0:K2, cs],
                            nstab[0:K2, cs], AL.mult)

                    items += [do_cast, do_tr, do_scale]
                    hoff += hs


                # drive stream (u-combos) for this chunk -> nbt cols 200:208
                tcu = tc_ if t0 + tc_ < T else tc_ - 1  # steps with a next-u
                if tcu > 0:
                    def do_u(ci=ci, t0=t0, tcu=tcu, cur=cur):
                        ut, ub = utile[cur], ubf[cur]
                        # utile[:, j*8+i] (i<7) = 0.8*u[t0+j] - u[t0+j+1]
                        o_ap = ut[:, 0:tcu * 8].rearrange("b (j i) -> b j i", i=8)[:, :, 0:7]
                        i0 = us[:, t0 * 7:(t0 + tcu) * 7].rearrange("b (j i) -> b j i", i=7)
                        i1 = us[:, (t0 + 1) * 7:(t0 + tcu + 1) * 7].rearrange("b (j i) -> b j i", i=7)
                        nc.vector.scalar_tensor_tensor(o_ap, i0, L, i1, AL.mult, AL.subtract)
                        nc.gpsimd.memset(
                            ut[:, 0:tcu * 8].rearrange("b (j i) -> b j i", i=8)[:, :, 7:8],
                            -ALPHA)
                        nc.gpsimd.tensor_tensor(ub[:, 0:tcu * 8], ut[:, 0:tcu * 8],
                                                ktab[:, 0:tcu * 8], AL.mult)
                        dst = nbt_d.ap()[t0 * B:(t0 + tcu) * B, 224:232].rearrange(
                            "(j b) i -> b j i", b=B)
                        nc.sync.dma_start(dst, ub[:, 0:tcu * 8])
                    items.insert(0, do_u)
                return items

            def post_pass_items(ci):
                """Reconstruction of chunk ci (after its scan): per batch row b,
                transpose S'-streams to natural (j,n), then cumsum via a
                lower-triangular-ones matmul (+rank-1 carry), scale-evac on ACT,
                z-dot on DVE, and contiguous DMAs out."""
                t0, tc_ = chunks[ci]
                cur = ci % 2
                last = ci == len(chunks) - 1
                items = []

                def do_s2tot(cur=cur, tc_=tc_):
                    cs = slice(0, tc_ * B)
                    nc.gpsimd.tensor_tensor(s2b[cur][0:K2, cs], s2b[cur][0:K2, cs],
                                            n2b[cur][0:K2, cs], AL.add)
                items.append(do_s2tot)

                s1v = s1b[cur][:, 0:tc_ * B].rearrange("p (j b) -> p b j", b=B)
                s2v = s2b[cur][0:K2, 0:tc_ * B].rearrange("p (j b) -> p b j", b=B)
                for b in range(B):
                    def do_block(b=b, t0=t0, tc_=tc_, ci=ci):
                        # S natural: (j, n) for this b
                        pst = pstbp.tile([128, N], BF16, tag="pstb")
                        nc.tensor.matmul(pst[0:tc_, 0:K1], s1v[:, b, :], id128b,
                                         is_transpose=True, start=True,
                                         stop=False)
                        nc.tensor.matmul(pst[0:tc_, K1:N], s2v[:, b, :],
                                         id128b[0:K2, 0:K2], is_transpose=True,
                                         start=False, stop=True)
                        sn = snatp.tile([128, N], BF16, tag="snat")
                        nc.scalar.activation(sn[0:tc_, :], pst[0:tc_, :], AF.Copy)
                        # cumsum + carry:  p2[j',n] = sum_{j<=j'} S[j,n] + 4*y_t0[n]
                        p2 = p2p.tile([128, N], FP32, tag="p2")
                        nc.tensor.matmul(p2[0:tc_, :], lt[0:tc_, 0:tc_],
                                         sn[0:tc_, :], start=True, stop=False)
                        nc.tensor.matmul(p2[0:tc_, :], c4[:, 0:tc_],
                                         ycar[:, b * N:(b + 1) * N],
                                         start=False, stop=True)
                        yn = ynatp.tile([128, N], FP32, tag="ynat")
                        nc.scalar.activation(yn[0:tc_, :], p2[0:tc_, :], AF.Copy,
                                             scale=scol[0:tc_, :])
                        nc.vector.scalar_tensor_tensor(
                            junk[0:tc_, :], yn[0:tc_, :], 1.0, wrep[0:tc_, :],
                            AL.mult, AL.mult,
                            accum_out=zblk[ci % 2][0:tc_, b:b + 1])
                        nc.sync.dma_start(ys_d.ap()[b, t0:t0 + tc_, :], yn[0:tc_, :])
                    items.append(do_block)

                def do_z(ci=ci, t0=t0, tc_=tc_):
                    psz = pstp.tile([128, N], FP32, tag="pst")
                    nc.tensor.matmul(psz[0:B, 0:tc_], zblk[ci % 2][0:tc_, :],
                                     id128[0:tc_, 0:tc_], is_transpose=True)
                    zt = ztp.tile([B, TC], FP32, tag="zt")
                    nc.scalar.activation(zt[:, 0:tc_], psz[0:B, 0:tc_],
                                         AF.Sigmoid, bias=_BOUT[0])
                    nc.sync.dma_start(
                        zs_d.ap()[:, t0:t0 + tc_, :].rearrange("b t o -> b (t o)"),
                        zt[:, 0:tc_])
                items.append(do_z)

                def do_carry(t0=t0, tc_=tc_):
                    # next chunk's carry: y_{t0+tc} = y_seq[t0+tc-1] (cast DMA)
                    nc.gpsimd.dma_start(ycar, ys_d.ap()[:, t0 + tc_ - 1, :])
                items.append(do_carry)

                if last:
                    def do_yf():
                        nc.sync.dma_start(yf_d.ap(), ys_d.ap()[:, T - 1, :])
                    items.append(do_yf)
                return items

            # ======== main emission loop ========
            pending = pre_pass_items(0)
            for it in pending:
                it()

            for ci, (t0, tc_) in enumerate(chunks):
                cur = ci % 2
                items = []
                if ci > 0:
                    items += post_pass_items(ci - 1)
                if ci + 1 < len(chunks):
                    items += pre_pass_items(ci + 1)

                if ci > 0:
                    ptc = chunks[ci - 1][1]
                    f = float(L ** ptc)
                    nc.vector.tensor_scalar_mul(psA[:], psA[:], f)
                    nc.vector.tensor_scalar_mul(psB[0:K2, :], psB[0:K2, :], f)

                ndone = 0
                for j in range(tc_):
                    t = t0 + j
                    js = slice(j * B, (j + 1) * B)
                    nc.vector.scalar_tensor_tensor(
                        s1b[cur][:, js], psA[:], 0.0, n1b[cur][:, js],
                        AL.max, AL.add)
                    nc.scalar.activation(s2b[cur][0:K2, js], psB[0:K2, :], AF.Relu)
                    if t < T - 1:
                        stop = t == T - 2
                        nc.tensor.matmul(psA[:], cb1a, s1b[cur][:, js],
                                         start=False, stop=False)
                        nc.tensor.matmul(psA[:], cb2a, s2b[cur][:, js],
                                         start=False, stop=False)
                        nc.tensor.matmul(psA[:], cb2a, n2b[cur][:, js],
                                         start=False, stop=stop)
                        nc.tensor.matmul(psB[0:K2, :], cb1b, s1b[cur][:, js],
                                         start=False, stop=False)
                        nc.tensor.matmul(psB[0:K2, :], cb2b, s2b[cur][:, js],
                                         start=False, stop=False)
                        nc.tensor.matmul(psB[0:K2, :], cb2b, n2b[cur][:, js],
                                         start=False, stop=stop)
                    # interleave deferred work
                    want = (j + 1) * len(items) // tc_
                    while ndone < want:
                        items[ndone]()
                        ndone += 1
                while ndone < len(items):
                    items[ndone]()
                    ndone += 1

            # final chunk's post pass
            for it in post_pass_items(len(chunks) - 1):
                it()

    nc.compile()
    return nc


_BOUT = [0.0]  # patched by kernel() before build (bias baked into program)


def _host_constants(W_in_raw, W_rec, b_rec, w_out):
    bf = ml_dtypes.bfloat16
    W_in = np.abs(W_in_raw.astype(np.float32))
    Wa = np.concatenate([W_in, b_rec.astype(np.float32)[:, None]], axis=1)  # (N,8)
    C = (0.25 * W_rec.astype(np.float32).T)  # [k, n]
    cb1a = C[0:K1, 0:K1].astype(bf)
    cb1b = C[0:K1, K1:N].astype(bf)
    cb2a = np.zeros((128, 128), bf)
    cb2b = np.zeros((128, K2), bf)
    cb2a[0:K2, :] = C[K1:N, 0:K1].astype(bf)
    cb2b[0:K2, :] = C[K1:N, K1:N].astype(bf)
    cb2a[96:104, :] = (-Wa.T[:, 0:K1]).astype(bf)
    cb2b[96:104, :] = (-Wa.T[:, K1:N]).astype(bf)

    j = np.arange(TC, dtype=np.float64)
    nst = (4.0 * NOISE_SCALE * L ** (-(j + 1.0))).astype(np.float32)
    nstab = np.broadcast_to(np.repeat(nst, B)[None, :], (128, TC * B)).astype(bf)
    ktab = np.broadcast_to(np.repeat((L ** (-(j + 1.0))).astype(np.float32), 8)[None, :],
                           (B, TC * 8)).astype(np.float32).copy()
    scolm = (0.25 * L ** (np.arange(128) + 1.0)).astype(np.float32)[:, None]
    ltm = np.tril(np.ones((128, 128), np.float32)).T.astype(bf)  # [l, j'] = l<=j'
    wrep = np.broadcast_to(w_out.astype(np.float32)[0][None, :], (128, N)).copy()
    return dict(cb1a=np.ascontiguousarray(cb1a), cb1b=np.ascontiguousarray(cb1b),
                cb2a=cb2a, cb2b=cb2b, nstab=np.ascontiguousarray(nstab),
                ktab=ktab, scol=np.ascontiguousarray(scolm), wrep=wrep,
                lt=np.ascontiguousarray(ltm))


_prog_cache = {}


def _get_program(T, bout):
    key = (T, float(bout))
    if key not in _prog_cache:
        _BOUT[0] = float(bout)
        _prog_cache[key] = build_program(T)
    return _prog_cache[key]


def kernel(y0, u_sequence, noise, W_in_raw, W_rec, b_rec, w_out, b_out,
           run_kwargs=None):
    T = u_sequence.shape[1]
    Bfull = y0.shape[0]
    assert Bfull == B * NCORES
    nc = _get_program(T, float(np.asarray(b_out).reshape(-1)[0]))
    consts = _host_constants(W_in_raw, W_rec, b_rec, w_out)

    in_maps = []
    for c in range(NCORES):
        sl = slice(c * B, (c + 1) * B)
        m = dict(consts)
        m["y0"] = np.ascontiguousarray(y0[sl].astype(np.float32))
        m["u"] = np.ascontiguousarray(u_sequence[sl].astype(np.float32))
        m["noise"] = np.ascontiguousarray(noise[sl].astype(np.float32))
        in_maps.append(m)

    res = bass_utils.run_bass_kernel_spmd(
        nc, in_maps, core_ids=list(range(NCORES)), **(run_kwargs or {}))

    y_seq = np.concatenate([res.results[c]["y_seq"] for c in range(NCORES)], axis=0)
    z_seq = np.concatenate([res.results[c]["z_seq"] for c in range(NCORES)], axis=0)
    y_fin = np.concatenate([res.results[c]["y_final"] for c in range(NCORES)], axis=0)
    kernel.last_result = res
    return y_seq, z_seq, y_fin
